# revision 6
# baseline (speedup 1.0000x reference)
"""nn_BackgroundLoss segment-reduce kernel for 8 Trainium2 NeuronCores.

Contract: kernel(**inputs) takes the FULL unsharded inputs (w, beta, x, y,
particle_id as numpy arrays; only beta/particle_id are used by the math) and
returns the full output (a float32 scalar), running the computation on the 8
NeuronCores via a Bass/Tile SPMD kernel.

Algorithm (exact segment max, segment-sharded):
  The loss needs seg_max[p] = max beta over hits of particle p (P=50000
  segments), the count of non-empty segments with p > 0, and the pid==0
  (noise) sum/count.  Segments are sharded across the 8 cores: core c owns
  hi-blocks [49c, 49c+49) where hi = pid >> 7, i.e. pids [6272c, 6272c+6272).
  While sharding, the host performs a pure layout permutation: each hit is
  placed at (partition = pid & 127, column = rank*49 + (hi - 49*core)) of a
  [128, Kp*49] fp16 tile initialised to -1 (rank = arrival index within the
  segment, Kp = max segment size).  Each (partition, col%49) cell then holds
  one segment spread over Kp strided slots, so the device computes the EXACT
  per-segment max with a log-tree of wide pairwise-max ops (fp16, 2x DVE
  throughput), and presence is simply seg_max > -0.5: empty segments and
  out-of-range pids never get a hit and stay at -1.  pid==0 hits are noise,
  not a segment; the host routes them to a dedicated [128, Kn] block at the
  tail of core 0's tile, where a masked sum/count yields the noise term.

  Per-core partials (n_valid, sum(1-seg_max), noise_sum, noise_cnt) are
  folded across partitions with a ones-matmul, AllReduced (add) over the 8
  cores, and every core finalises the scalar; the host returns core 0's y.
"""
import sys

if '/opt/trn_rl_repo' not in sys.path:
    sys.path.insert(0, '/opt/trn_rl_repo')

import numpy as np
from concourse import bacc, tile, mybir
from concourse.bass_utils import run_bass_kernel_spmd

F32 = mybir.dt.float32
F16 = mybir.dt.float16
Alu = mybir.AluOpType

SB = 0.1
NUM_PIDS = 50_000
N_CORES = 8
NCOL = 49          # hi-blocks per core; 49*8 = 392 >= ceil(50000/128) = 391
PAD = -1.0         # sentinel; real beta is in [0, 1)

_cache: dict = {}

# "cc":   device AllReduce of the [1,4] partials; core 0's y is the answer
# "cc2":  same + an early dummy AllReduce to absorb cc bootstrap latency
# "nocc": each core outputs its [1,4] partials; host sums and finalizes
MODE = "cc2"
NDMA = 4           # W is fetched in NDMA chunked dma_starts (parallel queues)


def _build(Kp: int, Kn: int, mode: str = "cc"):
    use_cc = mode != "nocc"
    nc = bacc.Bacc("TRN2", target_bir_lowering=False, debug=False,
                   num_devices=N_CORES)
    W_d = nc.dram_tensor("W", [128, Kp * NCOL + Kn], F16,
                         kind="ExternalInput").ap()
    y_d = nc.dram_tensor("y", [1, 1] if use_cc else [1, 4], F32,
                         kind="ExternalOutput").ap()

    with tile.TileContext(nc) as tc:
        with (
            tc.tile_pool(name="bulk", bufs=1) as bulkp,
            tc.tile_pool(name="fin", bufs=1) as finp,
            tc.tile_pool(name="psum", bufs=1, space="PSUM") as psump,
            tc.tile_pool(name="dram", bufs=1, space="DRAM") as dramp,
        ):
            if mode == "cc2":
                # dummy collective issued first: its channel-setup latency
                # overlaps the DMA + reduction phase below
                d_in = dramp.tile([1, 1], F32, tag="d_in")
                d_out = dramp.tile([1, 1], F32, tag="d_out")
                dz = finp.tile([1, 1], F32, tag="dz")
                nc.vector.memset(dz[:], 0.0)
                nc.sync.dma_start(out=d_in[:], in_=dz[:])
                nc.gpsimd.collective_compute(
                    "AllReduce", Alu.add,
                    replica_groups=[list(range(N_CORES))],
                    ins=[d_in.opt()],
                    outs=[d_out.opt()],
                )

            W = bulkp.tile([128, Kp * NCOL + Kn], F16, tag="W")
            tot = Kp * NCOL + Kn
            step = (tot + NDMA - 1) // NDMA
            for i in range(0, tot, step):
                j = min(i + step, tot)
                nc.sync.dma_start(out=W[:, i:j], in_=W_d[:, i:j])

            # exact per-segment max: pairwise-max tree over the Kp k-blocks
            cur, k = W, Kp
            lvl = 0
            while k > 1:
                if k % 2 == 1:
                    nc.vector.tensor_tensor(
                        cur[:, 0:NCOL], cur[:, 0:NCOL],
                        cur[:, (k - 1) * NCOL:k * NCOL], Alu.max)
                    k -= 1
                h = k // 2
                nxt = bulkp.tile([128, h * NCOL], F32 if h == 1 else F16,
                                 tag=f"lvl{lvl}")
                nc.vector.tensor_tensor(nxt[:], cur[:, 0:h * NCOL],
                                        cur[:, h * NCOL:k * NCOL], Alu.max)
                cur, k, lvl = nxt, h, lvl + 1
            seg = cur  # [128, 49] f32: seg_max, or -1 for empty/invalid

            # presence & attract partials
            pres = finp.tile([128, NCOL], F32, tag="pres")
            onem = finp.tile([128, NCOL], F32, tag="onem")
            pm = finp.tile([128, NCOL], F32, tag="pm")
            nc.vector.tensor_scalar(pres[:], seg[:], -0.5, None, Alu.is_gt)
            nc.vector.tensor_scalar(onem[:], seg[:], -1.0, 1.0, Alu.mult,
                                    Alu.add)
            nc.vector.tensor_mul(pm[:], pres[:], onem[:])

            # noise partials from the [128, Kn] tail block (core 0 only has
            # real hits there; other cores contribute zeros)
            Wn = W[:, Kp * NCOL:Kp * NCOL + Kn]
            nmask = finp.tile([128, Kn], F32, tag="nmask")
            nbeta = finp.tile([128, Kn], F32, tag="nbeta")
            nc.vector.tensor_scalar(nmask[:], Wn, -0.5, None, Alu.is_gt)
            nc.vector.tensor_scalar_max(nbeta[:], Wn, 0.0)

            S = finp.tile([128, 4], F32, tag="S")
            nc.vector.tensor_reduce(S[:, 0:1], pres[:], mybir.AxisListType.X,
                                    Alu.add)
            nc.vector.tensor_reduce(S[:, 1:2], pm[:], mybir.AxisListType.X,
                                    Alu.add)
            nc.vector.tensor_reduce(S[:, 2:3], nbeta[:], mybir.AxisListType.X,
                                    Alu.add)
            nc.vector.tensor_reduce(S[:, 3:4], nmask[:], mybir.AxisListType.X,
                                    Alu.add)

            # fold partitions: [1,4] = ones^T @ S
            ones = finp.tile([128, 1], F32, tag="ones")
            nc.vector.memset(ones[:], 1.0)
            red = psump.tile([1, 4], F32, tag="red")
            nc.tensor.matmul(red[:], ones[:], S[:], start=True, stop=True)
            F = finp.tile([1, 4], F32, tag="F")
            nc.vector.tensor_copy(F[:], red[:])

            if not use_cc:
                nc.sync.dma_start(out=y_d[:], in_=F[:])
            else:
                cc_in = dramp.tile([1, 4], F32, tag="cc_in")
                cc_out = dramp.tile([1, 4], F32, tag="cc_out")
                nc.sync.dma_start(out=cc_in[:], in_=F[:])
                nc.gpsimd.collective_compute(
                    "AllReduce", Alu.add,
                    replica_groups=[list(range(N_CORES))],
                    ins=[cc_in.opt()],
                    outs=[cc_out.opt()],
                )
                G = finp.tile([1, 4], F32, tag="G")
                nc.sync.dma_start(out=G[:], in_=cc_out[:])

                # y = G1/max(G0,1) + SB * G2/max(G3,1)
                a = finp.tile([1, 8], F32, tag="a")
                nc.vector.tensor_scalar_max(a[:, 0:1], G[:, 0:1], 1.0)
                nc.vector.tensor_scalar_max(a[:, 1:2], G[:, 3:4], 1.0)
                nc.vector.reciprocal(a[:, 2:3], a[:, 0:1])
                nc.vector.reciprocal(a[:, 3:4], a[:, 1:2])
                nc.vector.tensor_mul(a[:, 4:5], G[:, 1:2], a[:, 2:3])
                nc.vector.tensor_mul(a[:, 5:6], G[:, 2:3], a[:, 3:4])
                nc.vector.tensor_scalar(a[:, 6:7], a[:, 5:6], SB, None,
                                        Alu.mult)
                nc.vector.tensor_tensor(a[:, 7:8], a[:, 4:5], a[:, 6:7],
                                        Alu.add)
                nc.sync.dma_start(out=y_d[:], in_=a[:, 7:8])

    nc.compile()
    return nc


def _shard(beta: np.ndarray, pid: np.ndarray):
    """Layout permutation: route each hit to its segment's owner core and
    slot it at (row=pid&127, col=rank*49 + local_hi); pid==0 hits go to the
    noise tail block of core 0.  Returns per-core [128, Kp*49+Kn] fp16
    arrays (PAD = -1 in empty slots) and the shape key (Kp, Kn)."""
    n = beta.shape[0]
    counts = np.bincount(pid, minlength=NUM_PIDS)
    n0 = int(counts[0])
    Kmax = int(counts[1:].max())
    Kp = (Kmax + 7) // 8 * 8
    Kn = max(((n0 + 127) // 128 + 1) // 2 * 2, 2)

    # rank of each hit within its segment (arrival order)
    order = np.argsort(pid, kind="stable")
    starts = np.concatenate([[0], np.cumsum(counts)[:-1]])
    rank = np.empty(n, dtype=np.int64)
    rank[order] = np.arange(n, dtype=np.int64) - starts[pid[order]]

    W = np.full((N_CORES, 128, Kp * NCOL + Kn), PAD, dtype=np.float16)
    b16 = beta.astype(np.float16)

    m = pid > 0
    hi = pid[m] >> 7
    core = hi // NCOL
    col = hi - core * NCOL
    W[core, pid[m] & 127, rank[m] * NCOL + col] = b16[m]

    if n0:
        j = np.arange(n0, dtype=np.int64)
        W[0, j % 128, Kp * NCOL + j // 128] = b16[pid == 0]
    return W, (Kp, Kn)


def _postprocess(res):
    if MODE == "nocc":
        G = np.sum([res[c]["y"][0] for c in range(N_CORES)], axis=0,
                   dtype=np.float64)
        out = G[1] / max(G[0], 1.0) + SB * G[2] / max(G[3], 1.0)
    else:
        out = res[0]["y"][0, 0]
    return np.asarray(np.float32(out))


def kernel(w, beta, x, y, particle_id):
    beta = np.ascontiguousarray(np.asarray(beta, dtype=np.float32))
    pid = np.ascontiguousarray(np.asarray(particle_id, dtype=np.int32))

    W, key = _shard(beta, pid)
    ckey = key + (MODE,)
    if ckey not in _cache:
        _cache[ckey] = _build(*key, mode=MODE)
    nc = _cache[ckey]

    in_maps = [{"W": W[c]} for c in range(N_CORES)]
    res = run_bass_kernel_spmd(nc, in_maps, list(range(N_CORES))).results
    return _postprocess(res)


# revision 7
# speedup vs baseline: 4.1123x; 4.1123x over previous
"""nn_BackgroundLoss segment-reduce kernel for 8 Trainium2 NeuronCores.

Contract: kernel(**inputs) takes the FULL unsharded inputs (w, beta, x, y,
particle_id as numpy arrays; only beta/particle_id are used by the math) and
returns the full output (a float32 scalar), running the computation on the 8
NeuronCores via a Bass/Tile SPMD kernel.

Algorithm (exact segment max, segment-sharded):
  The loss needs seg_max[p] = max beta over hits of particle p (P=50000
  segments), the count of non-empty segments with p > 0, and the pid==0
  (noise) sum/count.  Segments are sharded across the 8 cores: core c owns
  hi-blocks [49c, 49c+49) where hi = pid >> 7, i.e. pids [6272c, 6272c+6272).
  While sharding, the host performs a pure layout permutation: each hit is
  placed at (partition = pid & 127, column = rank*49 + (hi - 49*core)) of a
  [128, Kp*49] fp16 tile initialised to -1 (rank = arrival index within the
  segment, Kp = max segment size).  Each (partition, col%49) cell then holds
  one segment spread over Kp strided slots, so the device computes the EXACT
  per-segment max with a log-tree of wide pairwise-max ops (fp16, 2x DVE
  throughput), and presence is simply seg_max > -0.5: empty segments and
  out-of-range pids never get a hit and stay at -1.  pid==0 hits are noise,
  not a segment; the host routes them to a dedicated [128, Kn] block at the
  tail of core 0's tile, where a masked sum/count yields the noise term.

  Per-core partials (n_valid, sum(1-seg_max), noise_sum, noise_cnt) are
  folded across partitions with a ones-matmul, AllReduced (add) over the 8
  cores, and every core finalises the scalar; the host returns core 0's y.
"""
import sys

if '/opt/trn_rl_repo' not in sys.path:
    sys.path.insert(0, '/opt/trn_rl_repo')

import numpy as np
from concourse import bacc, tile, mybir
from concourse.bass_utils import run_bass_kernel_spmd

F32 = mybir.dt.float32
F16 = mybir.dt.float16
Alu = mybir.AluOpType

SB = 0.1
NUM_PIDS = 50_000
N_CORES = 8
NCOL = 49          # hi-blocks per core; 49*8 = 392 >= ceil(50000/128) = 391
PAD = -1.0         # sentinel; real beta is in [0, 1)

_cache: dict = {}

# "cc":   device AllReduce of the [1,4] partials; core 0's y is the answer
# "cc2":  same + an early dummy AllReduce to absorb cc bootstrap latency
# "nocc": each core outputs its [1,4] partials; host sums and finalizes
MODE = "nocc"
NDMA = 4           # W is fetched in NDMA chunked dma_starts (parallel queues)


def _build(Kp: int, Kn: int, mode: str = "cc"):
    use_cc = mode != "nocc"
    nc = bacc.Bacc("TRN2", target_bir_lowering=False, debug=False,
                   num_devices=N_CORES)
    W_d = nc.dram_tensor("W", [128, Kp * NCOL + Kn], F16,
                         kind="ExternalInput").ap()
    y_d = nc.dram_tensor("y", [1, 1] if use_cc else [1, 4], F32,
                         kind="ExternalOutput").ap()

    with tile.TileContext(nc) as tc:
        with (
            tc.tile_pool(name="bulk", bufs=1) as bulkp,
            tc.tile_pool(name="fin", bufs=1) as finp,
            tc.tile_pool(name="psum", bufs=1, space="PSUM") as psump,
            tc.tile_pool(name="dram", bufs=1, space="DRAM") as dramp,
        ):
            if mode == "cc2":
                # dummy collective issued first: its channel-setup latency
                # overlaps the DMA + reduction phase below
                d_in = dramp.tile([1, 1], F32, tag="d_in")
                d_out = dramp.tile([1, 1], F32, tag="d_out")
                dz = finp.tile([1, 1], F32, tag="dz")
                nc.vector.memset(dz[:], 0.0)
                nc.sync.dma_start(out=d_in[:], in_=dz[:])
                nc.gpsimd.collective_compute(
                    "AllReduce", Alu.add,
                    replica_groups=[list(range(N_CORES))],
                    ins=[d_in.opt()],
                    outs=[d_out.opt()],
                )

            W = bulkp.tile([128, Kp * NCOL + Kn], F16, tag="W")
            tot = Kp * NCOL + Kn
            step = (tot + NDMA - 1) // NDMA
            for i in range(0, tot, step):
                j = min(i + step, tot)
                nc.sync.dma_start(out=W[:, i:j], in_=W_d[:, i:j])

            # exact per-segment max: pairwise-max tree over the Kp k-blocks
            cur, k = W, Kp
            lvl = 0
            while k > 1:
                if k % 2 == 1:
                    nc.vector.tensor_tensor(
                        cur[:, 0:NCOL], cur[:, 0:NCOL],
                        cur[:, (k - 1) * NCOL:k * NCOL], Alu.max)
                    k -= 1
                h = k // 2
                nxt = bulkp.tile([128, h * NCOL], F32 if h == 1 else F16,
                                 tag=f"lvl{lvl}")
                nc.vector.tensor_tensor(nxt[:], cur[:, 0:h * NCOL],
                                        cur[:, h * NCOL:k * NCOL], Alu.max)
                cur, k, lvl = nxt, h, lvl + 1
            seg = cur  # [128, 49] f32: seg_max, or -1 for empty/invalid

            # presence & attract partials
            pres = finp.tile([128, NCOL], F32, tag="pres")
            onem = finp.tile([128, NCOL], F32, tag="onem")
            pm = finp.tile([128, NCOL], F32, tag="pm")
            nc.vector.tensor_scalar(pres[:], seg[:], -0.5, None, Alu.is_gt)
            nc.vector.tensor_scalar(onem[:], seg[:], -1.0, 1.0, Alu.mult,
                                    Alu.add)
            nc.vector.tensor_mul(pm[:], pres[:], onem[:])

            # noise partials from the [128, Kn] tail block (core 0 only has
            # real hits there; other cores contribute zeros)
            Wn = W[:, Kp * NCOL:Kp * NCOL + Kn]
            nmask = finp.tile([128, Kn], F32, tag="nmask")
            nbeta = finp.tile([128, Kn], F32, tag="nbeta")
            nc.vector.tensor_scalar(nmask[:], Wn, -0.5, None, Alu.is_gt)
            nc.vector.tensor_scalar_max(nbeta[:], Wn, 0.0)

            S = finp.tile([128, 4], F32, tag="S")
            nc.vector.tensor_reduce(S[:, 0:1], pres[:], mybir.AxisListType.X,
                                    Alu.add)
            nc.vector.tensor_reduce(S[:, 1:2], pm[:], mybir.AxisListType.X,
                                    Alu.add)
            nc.vector.tensor_reduce(S[:, 2:3], nbeta[:], mybir.AxisListType.X,
                                    Alu.add)
            nc.vector.tensor_reduce(S[:, 3:4], nmask[:], mybir.AxisListType.X,
                                    Alu.add)

            # fold partitions: [1,4] = ones^T @ S
            ones = finp.tile([128, 1], F32, tag="ones")
            nc.vector.memset(ones[:], 1.0)
            red = psump.tile([1, 4], F32, tag="red")
            nc.tensor.matmul(red[:], ones[:], S[:], start=True, stop=True)
            F = finp.tile([1, 4], F32, tag="F")
            nc.vector.tensor_copy(F[:], red[:])

            if not use_cc:
                nc.sync.dma_start(out=y_d[:], in_=F[:])
            else:
                cc_in = dramp.tile([1, 4], F32, tag="cc_in")
                cc_out = dramp.tile([1, 4], F32, tag="cc_out")
                nc.sync.dma_start(out=cc_in[:], in_=F[:])
                nc.gpsimd.collective_compute(
                    "AllReduce", Alu.add,
                    replica_groups=[list(range(N_CORES))],
                    ins=[cc_in.opt()],
                    outs=[cc_out.opt()],
                )
                G = finp.tile([1, 4], F32, tag="G")
                nc.sync.dma_start(out=G[:], in_=cc_out[:])

                # y = G1/max(G0,1) + SB * G2/max(G3,1)
                a = finp.tile([1, 8], F32, tag="a")
                nc.vector.tensor_scalar_max(a[:, 0:1], G[:, 0:1], 1.0)
                nc.vector.tensor_scalar_max(a[:, 1:2], G[:, 3:4], 1.0)
                nc.vector.reciprocal(a[:, 2:3], a[:, 0:1])
                nc.vector.reciprocal(a[:, 3:4], a[:, 1:2])
                nc.vector.tensor_mul(a[:, 4:5], G[:, 1:2], a[:, 2:3])
                nc.vector.tensor_mul(a[:, 5:6], G[:, 2:3], a[:, 3:4])
                nc.vector.tensor_scalar(a[:, 6:7], a[:, 5:6], SB, None,
                                        Alu.mult)
                nc.vector.tensor_tensor(a[:, 7:8], a[:, 4:5], a[:, 6:7],
                                        Alu.add)
                nc.sync.dma_start(out=y_d[:], in_=a[:, 7:8])

    nc.compile()
    return nc


def _shard(beta: np.ndarray, pid: np.ndarray):
    """Layout permutation: route each hit to its segment's owner core and
    slot it at (row=pid&127, col=rank*49 + local_hi); pid==0 hits go to the
    noise tail block of core 0.  Returns per-core [128, Kp*49+Kn] fp16
    arrays (PAD = -1 in empty slots) and the shape key (Kp, Kn)."""
    n = beta.shape[0]
    counts = np.bincount(pid, minlength=NUM_PIDS)
    n0 = int(counts[0])
    Kmax = int(counts[1:].max())
    Kp = (Kmax + 7) // 8 * 8
    Kn = max(((n0 + 127) // 128 + 1) // 2 * 2, 2)

    # rank of each hit within its segment (arrival order)
    order = np.argsort(pid, kind="stable")
    starts = np.concatenate([[0], np.cumsum(counts)[:-1]])
    rank = np.empty(n, dtype=np.int64)
    rank[order] = np.arange(n, dtype=np.int64) - starts[pid[order]]

    W = np.full((N_CORES, 128, Kp * NCOL + Kn), PAD, dtype=np.float16)
    b16 = beta.astype(np.float16)

    m = pid > 0
    hi = pid[m] >> 7
    core = hi // NCOL
    col = hi - core * NCOL
    W[core, pid[m] & 127, rank[m] * NCOL + col] = b16[m]

    if n0:
        j = np.arange(n0, dtype=np.int64)
        W[0, j % 128, Kp * NCOL + j // 128] = b16[pid == 0]
    return W, (Kp, Kn)


def _postprocess(res):
    if MODE == "nocc":
        G = np.sum([res[c]["y"][0] for c in range(N_CORES)], axis=0,
                   dtype=np.float64)
        out = G[1] / max(G[0], 1.0) + SB * G[2] / max(G[3], 1.0)
    else:
        out = res[0]["y"][0, 0]
    return np.asarray(np.float32(out))


def kernel(w, beta, x, y, particle_id):
    beta = np.ascontiguousarray(np.asarray(beta, dtype=np.float32))
    pid = np.ascontiguousarray(np.asarray(particle_id, dtype=np.int32))

    W, key = _shard(beta, pid)
    ckey = key + (MODE,)
    if ckey not in _cache:
        _cache[ckey] = _build(*key, mode=MODE)
    nc = _cache[ckey]

    in_maps = [{"W": W[c]} for c in range(N_CORES)]
    res = run_bass_kernel_spmd(nc, in_maps, list(range(N_CORES))).results
    return _postprocess(res)
